# revision 3
# baseline (speedup 1.0000x reference)
"""Trainium2 Bass kernel for nn_DistanceConstraint.

loss = sum_{b,i,j} m_i m_j [cdist_ij < 10] relu(||e^_i - e^_j|| - 1) / (count + 1e-8)

Math used on-device (per batch b, one NeuronCore per batch):
  - e^ = e / ||e||  (row L2 normalization; norms ~22 so the 1e-12 eps clamp
    never binds); then ||e^_i - e^_j||^2 = 2 - 2 G_ij with G = E^ E^^T.
  - relu(sqrt(max(d2,0)) - 1) == sqrt(max(d2,1)) - 1 == sqrt(relu(1-2G) + 1) - 1
  - [cdist < 10] == [cd2 < 100] with cd2 computed by one augmented K=5 matmul:
    rows (cx,cy,cz,csq,1) x (-2cx,-2cy,-2cz,1,csq).
  - per-row partial sums come out of the fused accum_out of the final DVE op;
    the m_i weighting, cross-core sum and the final divide happen on host in
    float64 (exact for the tiny [128,64]-per-core partials).

Per [128,512] output tile: 4 fp32r Gram matmuls + 1 coord matmul (PE),
relu/sqrt (ACT), compare*mask and (s-1)*c with fused row-sum (DVE).
"""

import os
import numpy as np

B, N, D = 8, 2048, 512
NB = N // 128      # 16 row blocks
NCH = N // 512     # 4 column chunks
N_CORES = 8

_CACHE = {}
LAST_EXEC_NS = None


def _build():
    import concourse.bacc as bacc
    import concourse.mybir as mybir
    from concourse import tile

    dt = mybir.dt
    AF = mybir.ActivationFunctionType
    ALU = mybir.AluOpType
    f32 = dt.float32
    f32r = dt.float32r

    nc = bacc.Bacc("TRN2", target_bir_lowering=False, debug=False,
                   num_devices=N_CORES)
    emb = nc.dram_tensor("emb", [N, D], f32, kind="ExternalInput").ap()
    lmat = nc.dram_tensor("lmat", [5, N], f32, kind="ExternalInput").ap()
    rmat = nc.dram_tensor("rmat", [5, N], f32, kind="ExternalInput").ap()
    mbc = nc.dram_tensor("mbc", [128, N], f32, kind="ExternalInput").ap()
    iden = nc.dram_tensor("iden", [128, 128], f32, kind="ExternalInput").ap()
    accd = nc.dram_tensor("acc", [128, NB * NCH], f32, kind="ExternalOutput").ap()

    with tile.TileContext(nc) as tc:
        with tc.tile_pool(name="persist", bufs=1) as pp:
            XT = [pp.tile([128, N], f32r, tag=f"xt{k}", name=f"xt{k}")
                  for k in range(4)]
            Lst = pp.tile([5, N], f32, tag="lst")
            Rst = pp.tile([5, N], f32, tag="rst")
            Lt = pp.tile([5, N], f32r, tag="lmat")
            Rt = pp.tile([5, N], f32r, tag="rmat")
            Mb = pp.tile([128, N], f32, tag="mbc")
            Id = pp.tile([128, 128], f32, tag="iden")
            Sq = pp.tile([128, NB], f32, tag="sq")
            Nrm = pp.tile([128, NB], f32, tag="nrm")
            Invn = pp.tile([128, NB], f32, tag="invn")
            Acc = pp.tile([128, NB * NCH], f32, tag="acc")

            nc.sync.dma_start(Lst[:], lmat[:])
            nc.sync.dma_start(Rst[:], rmat[:])
            nc.vector.tensor_copy(Lt[:], Lst[:])
            nc.vector.tensor_copy(Rt[:], Rst[:])
            nc.sync.dma_start(Mb[:], mbc[:])
            nc.sync.dma_start(Id[:], iden[:])

            # ---- preprocessing: load, row-normalize, transpose to XT ----
            with (
                tc.tile_pool(name="pre", bufs=3) as pre,
                tc.tile_pool(name="pre_ps", bufs=1, space="PSUM") as pps,
            ):
                ptr = [None] * 4
                for b in range(NB):
                    xb = pre.tile([128, D], f32, tag="xb")
                    nc.sync.dma_start(xb[:], emb[128 * b:128 * (b + 1), :])
                    scr = pre.tile([128, D], f32, tag="scr")
                    nc.scalar.activation(scr[:], xb[:], AF.Square,
                                         accum_out=Sq[:, b:b + 1])
                    nc.scalar.activation(Nrm[:, b:b + 1], Sq[:, b:b + 1], AF.Sqrt)
                    nc.vector.reciprocal(Invn[:, b:b + 1], Nrm[:, b:b + 1])
                    xn = pre.tile([128, D], f32, tag="xn")
                    nc.vector.tensor_scalar(xn[:], xb[:], Invn[:, b:b + 1], None,
                                            op0=ALU.mult)
                    if b % 4 == 0:
                        ptr = [pps.tile([128, 512], f32, tag=f"tr{k}", name=f"tr{k}")
                               for k in range(4)]
                    o = 128 * (b % 4)
                    for k in range(4):
                        nc.tensor.transpose(ptr[k][:, o:o + 128],
                                            xn[:, 128 * k:128 * (k + 1)], Id[:])
                    if b % 4 == 3:
                        g = b // 4
                        for k in range(4):
                            dst = XT[k][:, 512 * g:512 * (g + 1)]
                            if (g + k) % 2 == 0:
                                nc.vector.tensor_copy(dst, ptr[k][:])
                            else:
                                nc.scalar.activation(dst, ptr[k][:], AF.Copy)

            # ---- main loop over 16x4 output tiles ----
            with (
                tc.tile_pool(name="ps_e", bufs=2, space="PSUM") as ppe,
                tc.tile_pool(name="ps_c", bufs=2, space="PSUM") as ppc,
                tc.tile_pool(name="mwork", bufs=3) as mw,
            ):
                for r in range(NB):
                    for c in range(NCH):
                        t = NCH * r + c
                        pe_t = ppe.tile([128, 512], f32, tag="pe")
                        for k in range(4):
                            nc.tensor.matmul(
                                pe_t[:],
                                XT[k][:, 128 * r:128 * (r + 1)],
                                XT[k][:, 512 * c:512 * (c + 1)],
                                start=(k == 0), stop=(k == 3))
                        pc_t = ppc.tile([128, 512], f32, tag="pc")
                        nc.tensor.matmul(
                            pc_t[:],
                            Lt[:, 128 * r:128 * (r + 1)],
                            Rt[:, 512 * c:512 * (c + 1)],
                            start=True, stop=True)
                        # r1 = relu(1 - 2G) = relu(d2 - 1)
                        r1 = mw.tile([128, 512], f32, tag="r1")
                        nc.scalar.activation(r1[:], pe_t[:], AF.Relu,
                                             bias=1.0, scale=-2.0)
                        # s = sqrt(r1 + 1) = sqrt(max(d2, 1))
                        s = mw.tile([128, 512], f32, tag="s")
                        nc.scalar.activation(s[:], r1[:], AF.Sqrt, bias=1.0)
                        # c = (cd2 < 100) * m_j
                        cm = mw.tile([128, 512], f32, tag="cm")
                        nc.vector.scalar_tensor_tensor(
                            cm[:], pc_t[:], 100.0,
                            Mb[:, 512 * c:512 * (c + 1)],
                            op0=ALU.is_lt, op1=ALU.mult)
                        # y = (s - 1) * c ; acc[:, t] = rowsum(y)
                        y = mw.tile([128, 512], f32, tag="y")
                        nc.vector.scalar_tensor_tensor(
                            y[:], s[:], -1.0, cm[:],
                            op0=ALU.add, op1=ALU.mult,
                            accum_out=Acc[:, t:t + 1])
                nc.sync.dma_start(accd[:], Acc[:])

    nc.compile()
    return nc


def _get_nc():
    if "nc" not in _CACHE:
        _CACHE["nc"] = _build()
    return _CACHE["nc"]


def kernel(embeddings, coords, mask):
    global LAST_EXEC_NS
    from concourse.bass_utils import run_bass_kernel_spmd

    nc = _get_nc()
    embeddings = np.asarray(embeddings)
    coords = np.asarray(coords)
    mask = np.asarray(mask)

    iden = np.eye(128, dtype=np.float32)
    ones = np.ones(N, np.float32)
    in_maps = []
    for b in range(B):
        c = np.ascontiguousarray(coords[b].astype(np.float32))
        csq = (c * c).sum(-1).astype(np.float32)
        L = np.ascontiguousarray(
            np.stack([c[:, 0], c[:, 1], c[:, 2], csq, ones]).astype(np.float32))
        R = np.ascontiguousarray(
            np.stack([-2 * c[:, 0], -2 * c[:, 1], -2 * c[:, 2], ones,
                      csq]).astype(np.float32))
        mb = np.ascontiguousarray(
            np.broadcast_to(mask[b].astype(np.float32), (128, N)))
        in_maps.append({
            "emb": np.ascontiguousarray(embeddings[b].astype(np.float32)),
            "lmat": L, "rmat": R, "mbc": mb, "iden": iden,
        })

    res = run_bass_kernel_spmd(nc, in_maps, list(range(N_CORES)))
    LAST_EXEC_NS = res.exec_time_ns

    num = 0.0
    for b in range(B):
        acc = res.results[b]["acc"].astype(np.float64)       # [128, 64]
        r = acc.reshape(128, NB, NCH).sum(-1)                # [p, rb]
        mi = mask[b].astype(np.float64).reshape(NB, 128).T   # [p, rb]
        num += float((r * mi).sum())
    cnt = sum(float(mask[b].astype(np.float64).sum()) ** 2 for b in range(B))
    out = np.asarray(np.float32(num / (cnt + 1e-8)))
    return out
